# revision 38
# baseline (speedup 1.0000x reference)
"""Trainium2 kernel for nn_MiddleHeadLayer: 2-layer tanh MLP + row-dot + sigmoid.

    inner = tanh(batch @ W1.T + b1)        batch [N, 1024], W1 [4096, 1024]
    wx    = tanh(inner @ W2.T + b2)        W2 [1024, 4096]
    out   = sigmoid(sum(wx * batch, -1))   [N]

Data-parallel over 8 NeuronCores: each core handles N/8 = 2048 rows;
weights replicated, resident in SBUF as fp16 (f32 weights do not fit in
24MB SBUF; fp16 matmuls run at full PE rate and keep absmax error ~3e-3).

Per-core dataflow, in blocks of R=256 rows:
  phase 1: innerT[dff, rows] = tanh(W1T.T @ batchT + b1) — stationary W1T
           chunks [128,128], moving batchT [128, R], fp16 in / f32 PSUM,
           ACT applies the per-partition (d_ff) bias and writes fp16.
  phase 2: wx[rows, dmodel] = tanh(innerT.T @ W2T + b2) — stationary innerT
           chunks, moving W2T [128, 512]. b2 (free-dim bias) is folded in
           as a rank-1 ones x b2 matmul into the same PSUM group.
  dot:     z[rows] = sum(wx * batch_f32) via fused DVE tensor_tensor_reduce
           along the free dim; sigmoid once at the end on all z columns.
"""

from contextlib import ExitStack

import numpy as np
import orjson

import concourse.bass as bass
import concourse.tile as tile
from concourse import mybir
from concourse import bass_utils

D_MODEL = 1024
D_FF = 4096
N_TOTAL = 16384
N_CORES = 8
NC_ROWS = N_TOTAL // N_CORES          # 2048 rows per core
R = 256                               # row-block size
N_BLOCKS = NC_ROWS // R               # 8
K1 = D_MODEL // 128                   # 8 contraction chunks for matmul1
M1 = D_FF // 128                      # 32 d_ff chunks
RG = R // 128                         # row groups per block
NH = D_MODEL // 512                   # d_model halves for phase 2
F16 = mybir.dt.float16
F32 = mybir.dt.float32


# ---------------------------------------------------------------------------
# This walrus build rejects >2 sem waits on a single instruction, while Tile's
# wait assignment freely attaches more (e.g. the exit drain gets one wait per
# outstanding logical proc). Legalize at the BIR-JSON level: hoist excess
# waits onto EventSemaphore instructions inserted directly before the
# offending instruction on the same engine stream (identical semantics).
MAX_WAITS = 1


def _legalize_sync_waits(bir: dict) -> dict:
    ctr = 0
    for fn in bir.get("functions", []):
        for blk in fn.get("blocks", []):
            insts = blk.get("instructions")
            if not insts:
                continue
            out = []
            changed = False
            for inst in insts:
                si = inst.get("sync_info")
                ow = (si or {}).get("on_wait") or []
                limit = 2 if inst.get("opcode") == "EventSemaphore" else MAX_WAITS
                if len(ow) > limit:
                    changed = True
                    excess, keep = ow[:-limit], ow[-limit:]
                    for i in range(0, len(excess), MAX_WAITS):
                        ctr += 1
                        out.append({
                            "debug": inst.get("debug"),
                            "engine": inst["engine"],
                            "ins": [],
                            "outs": [],
                            "name": f"legalwait-{ctr}",
                            "opcode": "EventSemaphore",
                            "sync_info": {
                                "on_update": [],
                                "on_wait": excess[i:i + MAX_WAITS],
                            },
                        })
                    si["on_wait"] = keep
                out.append(inst)
            if changed:
                blk["instructions"] = out
    return bir


_orig_to_json_bytes = bass.Bass.to_json_bytes


def _patched_to_json_bytes(self) -> bytes:
    return orjson.dumps(_legalize_sync_waits(orjson.loads(_orig_to_json_bytes(self))))


bass.Bass.to_json_bytes = _patched_to_json_bytes


def build_bass(n_blocks=N_BLOCKS):
    nc = bass.Bass("TRN2", target_bir_lowering=False, debug=False)

    w1t_d = nc.dram_tensor("w1t", [D_MODEL, D_FF], F16, kind="ExternalInput")
    w2t_d = nc.dram_tensor("w2t", [D_FF, D_MODEL], F16, kind="ExternalInput")
    b1_d = nc.dram_tensor("b1c", [128, M1], F32, kind="ExternalInput")
    b2_d = nc.dram_tensor("b2c", [1, D_MODEL], F16, kind="ExternalInput")
    ones_d = nc.dram_tensor("ones", [1, 128], F16, kind="ExternalInput")
    bt_d = nc.dram_tensor("batcht", [D_MODEL, NC_ROWS], F16, kind="ExternalInput")
    b_d = nc.dram_tensor("batch", [NC_ROWS, D_MODEL], F32, kind="ExternalInput")
    out_d = nc.dram_tensor("out", [NC_ROWS, 1], F32, kind="ExternalOutput")

    n_groups = n_blocks * RG
    W1CB = 4                       # w1t column blocks (of 1024 d_ff each)

    with tile.TileContext(nc) as tc, ExitStack() as ctx:
        wpool = ctx.enter_context(tc.tile_pool(name="weights", bufs=1))
        btpool = ctx.enter_context(tc.tile_pool(name="batchT", bufs=16))
        ipool = ctx.enter_context(tc.tile_pool(name="innerT", bufs=36))
        bfpool = ctx.enter_context(tc.tile_pool(name="batchf", bufs=4))
        wxpool = ctx.enter_context(tc.tile_pool(name="wx", bufs=4))
        spool = ctx.enter_context(tc.tile_pool(name="scratch", bufs=2))
        zpool = ctx.enter_context(tc.tile_pool(name="z", bufs=1))
        psum1 = ctx.enter_context(tc.tile_pool(name="psum1", bufs=3, space="PSUM"))
        psum2 = ctx.enter_context(tc.tile_pool(name="psum2", bufs=3, space="PSUM"))

        # DMA emission order = queue order: block-0 activations and the first
        # w1t column block go first so PE can start ~15us in; the remaining
        # weight bulk streams behind them.
        ones = wpool.tile([1, 128], F16, tag="ones")
        nc.sync.dma_start(ones[:], ones_d.ap()[:])

        bt0 = []
        for k in range(K1):
            t = btpool.tile([128, R], F16, tag="bt")
            nc.sync.dma_start(t[:], bt_d.ap()[k * 128:(k + 1) * 128, 0:R])
            bt0.append(t)

        CBW = D_FF // W1CB
        w1t = [[None] * W1CB for _ in range(K1)]
        for k in range(K1):
            t = wpool.tile([128, CBW], F16, tag=f"w1t{k}c0")
            nc.sync.dma_start(t[:], w1t_d.ap()[k * 128:(k + 1) * 128, 0:CBW])
            w1t[k][0] = t

        b1t = wpool.tile([128, M1], F32, tag="b1t")
        nc.sync.dma_start(b1t[:], b1_d.ap()[:])
        b2t = wpool.tile([1, D_MODEL], F16, tag="b2t")
        nc.sync.dma_start(b2t[:], b2_d.ap()[:])

        # rest of W1T column blocks
        for cb in range(1, W1CB):
            for k in range(K1):
                t = wpool.tile([128, CBW], F16, tag=f"w1t{k}c{cb}")
                nc.sync.dma_start(
                    t[:], w1t_d.ap()[k * 128:(k + 1) * 128, cb * CBW:(cb + 1) * CBW]
                )
                w1t[k][cb] = t
        # W2T chunks (first needed ~45us in, at phase 2 of block 0)
        w2t = []
        for m in range(M1):
            t = wpool.tile([128, D_MODEL], F16, tag=f"w2t{m}")
            nc.sync.dma_start(t[:], w2t_d.ap()[m * 128:(m + 1) * 128, :])
            w2t.append(t)

        z_all = zpool.tile([128, n_groups], F32)
        sig = zpool.tile([128, n_groups], F32, tag="sig")

        for b in range(n_blocks):
            # batchT chunks for this row block
            if b == 0:
                bt = bt0
            else:
                bt = []
                for k in range(K1):
                    t = btpool.tile([128, R], F16, tag="bt")
                    nc.sync.dma_start(
                        t[:], bt_d.ap()[k * 128:(k + 1) * 128, b * R:(b + 1) * R]
                    )
                    bt.append(t)

            # phase 1: innerT chunks [128 dff, R rows]
            it = []
            for m in range(M1):
                cb, mo = divmod(m, CBW // 128)
                ps = psum1.tile([128, R], F32)
                for k in range(K1):
                    nc.tensor.matmul(
                        ps[:],
                        w1t[k][cb][:, mo * 128:(mo + 1) * 128],
                        bt[k][:],
                        start=(k == 0),
                        stop=(k == K1 - 1),
                    )
                t = ipool.tile([128, R], F16, tag="it")
                nc.scalar.activation(
                    t[:], ps[:], mybir.ActivationFunctionType.Tanh,
                    bias=b1t[:, m:m + 1],
                )
                it.append(t)

            # phase 2 + row-dot per 128-row group
            for rg in range(RG):
                g = b * RG + rg
                bf = bfpool.tile([128, D_MODEL], F32, tag="bf")
                nc.sync.dma_start(
                    bf[:], b_d.ap()[g * 128:(g + 1) * 128, :]
                )
                wx = wxpool.tile([128, D_MODEL], F32, tag="wx")
                for h in range(NH):
                    ps2 = psum2.tile([128, 512], F32)
                    nc.tensor.matmul(
                        ps2[:],
                        ones[:],
                        b2t[:, h * 512:(h + 1) * 512],
                        start=True,
                        stop=False,
                    )
                    for m in range(M1):
                        nc.tensor.matmul(
                            ps2[:],
                            it[m][:, rg * 128:(rg + 1) * 128],
                            w2t[m][:, h * 512:(h + 1) * 512],
                            start=False,
                            stop=(m == M1 - 1),
                        )
                    nc.scalar.activation(
                        wx[:, h * 512:(h + 1) * 512], ps2[:],
                        mybir.ActivationFunctionType.Tanh,
                    )
                # z[g] = sum(wx * batch) along d_model, fused mult+reduce on DVE
                scratch = spool.tile([128, D_MODEL], F32, tag="scr")
                nc.vector.scalar_tensor_tensor(
                    out=scratch[:],
                    in0=wx[:],
                    scalar=1.0,
                    in1=bf[:],
                    op0=mybir.AluOpType.mult,
                    op1=mybir.AluOpType.mult,
                    accum_out=z_all[:, g:g + 1],
                )
                nc.scalar.activation(
                    sig[:, g:g + 1], z_all[:, g:g + 1],
                    mybir.ActivationFunctionType.Sigmoid,
                )
                nc.sync.dma_start(
                    out_d.ap()[g * 128:(g + 1) * 128, :], sig[:, g:g + 1]
                )

    return nc


_CACHED = {}


def _get_nc(n_blocks=N_BLOCKS):
    if n_blocks not in _CACHED:
        _CACHED[n_blocks] = build_bass(n_blocks)
    return _CACHED[n_blocks]


def _prep_in_maps(batch, W1, b1, W2, b2):
    batch = np.ascontiguousarray(batch, dtype=np.float32)
    w1t = np.ascontiguousarray(W1.T, dtype=np.float16)      # [1024, 4096]
    w2t = np.ascontiguousarray(W2.T, dtype=np.float16)      # [4096, 1024]
    # b1 as [128, 32]: column m holds b1[m*128:(m+1)*128] (per-partition bias)
    b1c = np.ascontiguousarray(
        np.asarray(b1, dtype=np.float32).reshape(M1, 128).T
    )
    b2c = np.ascontiguousarray(b2, dtype=np.float16).reshape(1, D_MODEL)
    ones = np.ones((1, 128), dtype=np.float16)
    batcht = np.ascontiguousarray(batch.T.astype(np.float16))  # [1024, 16384]

    in_maps = []
    for c in range(N_CORES):
        r0, r1 = c * NC_ROWS, (c + 1) * NC_ROWS
        in_maps.append({
            "w1t": w1t,
            "w2t": w2t,
            "b1c": b1c,
            "b2c": b2c,
            "ones": ones,
            "batcht": np.ascontiguousarray(batcht[:, r0:r1]),
            "batch": np.ascontiguousarray(batch[r0:r1]),
        })
    return in_maps


def kernel(batch, W1, b1, W2, b2, _trace=False, _trace_kwargs=None):
    in_maps = _prep_in_maps(batch, W1, b1, W2, b2)
    nc = _get_nc()
    res = bass_utils.run_bass_kernel_spmd(
        nc, in_maps, core_ids=list(range(N_CORES)),
        trace=_trace, **(_trace_kwargs or {}),
    )
    out = np.concatenate([res.results[c]["out"][:, 0] for c in range(N_CORES)])
    if _trace:
        return out, res
    return out


# revision 39
# speedup vs baseline: 1.0043x; 1.0043x over previous
"""Trainium2 kernel for nn_MiddleHeadLayer: 2-layer tanh MLP + row-dot + sigmoid.

    inner = tanh(batch @ W1.T + b1)        batch [N, 1024], W1 [4096, 1024]
    wx    = tanh(inner @ W2.T + b2)        W2 [1024, 4096]
    out   = sigmoid(sum(wx * batch, -1))   [N]

Data-parallel over 8 NeuronCores: each core handles N/8 = 2048 rows;
weights replicated, resident in SBUF as fp16 (f32 weights do not fit in
24MB SBUF; fp16 matmuls run at full PE rate and keep absmax error ~3e-3).

Per-core dataflow, in blocks of R=256 rows:
  phase 1: innerT[dff, rows] = tanh(W1T.T @ batchT + b1) — stationary W1T
           chunks [128,128], moving batchT [128, R], fp16 in / f32 PSUM,
           ACT applies the per-partition (d_ff) bias and writes fp16.
  phase 2: wx[rows, dmodel] = tanh(innerT.T @ W2T + b2) — stationary innerT
           chunks, moving W2T [128, 512]. b2 (free-dim bias) is folded in
           as a rank-1 ones x b2 matmul into the same PSUM group.
  dot:     z[rows] = sum(wx * batch_f32) via fused DVE tensor_tensor_reduce
           along the free dim; sigmoid once at the end on all z columns.
"""

from contextlib import ExitStack

import numpy as np
import orjson

import concourse.bass as bass
import concourse.tile as tile
from concourse import mybir
from concourse import bass_utils

D_MODEL = 1024
D_FF = 4096
N_TOTAL = 16384
N_CORES = 8
NC_ROWS = N_TOTAL // N_CORES          # 2048 rows per core
R = 256                               # row-block size
N_BLOCKS = NC_ROWS // R               # 8
K1 = D_MODEL // 128                   # 8 contraction chunks for matmul1
M1 = D_FF // 128                      # 32 d_ff chunks
RG = R // 128                         # row groups per block
NH = D_MODEL // 512                   # d_model halves for phase 2
F16 = mybir.dt.float16
F32 = mybir.dt.float32


# ---------------------------------------------------------------------------
# This walrus build rejects >2 sem waits on a single instruction, while Tile's
# wait assignment freely attaches more (e.g. the exit drain gets one wait per
# outstanding logical proc). Legalize at the BIR-JSON level: hoist excess
# waits onto EventSemaphore instructions inserted directly before the
# offending instruction on the same engine stream (identical semantics).
MAX_WAITS = 1


def _legalize_sync_waits(bir: dict) -> dict:
    ctr = 0
    for fn in bir.get("functions", []):
        for blk in fn.get("blocks", []):
            insts = blk.get("instructions")
            if not insts:
                continue
            out = []
            changed = False
            for inst in insts:
                si = inst.get("sync_info")
                ow = (si or {}).get("on_wait") or []
                limit = 2 if inst.get("opcode") == "EventSemaphore" else MAX_WAITS
                if len(ow) > limit:
                    changed = True
                    excess, keep = ow[:-limit], ow[-limit:]
                    for i in range(0, len(excess), MAX_WAITS):
                        ctr += 1
                        out.append({
                            "debug": inst.get("debug"),
                            "engine": inst["engine"],
                            "ins": [],
                            "outs": [],
                            "name": f"legalwait-{ctr}",
                            "opcode": "EventSemaphore",
                            "sync_info": {
                                "on_update": [],
                                "on_wait": excess[i:i + MAX_WAITS],
                            },
                        })
                    si["on_wait"] = keep
                out.append(inst)
            if changed:
                blk["instructions"] = out
    return bir


_orig_to_json_bytes = bass.Bass.to_json_bytes


def _patched_to_json_bytes(self) -> bytes:
    return orjson.dumps(_legalize_sync_waits(orjson.loads(_orig_to_json_bytes(self))))


bass.Bass.to_json_bytes = _patched_to_json_bytes


def build_bass(n_blocks=N_BLOCKS):
    nc = bass.Bass("TRN2", target_bir_lowering=False, debug=False)

    w1t_d = nc.dram_tensor("w1t", [D_MODEL, D_FF], F16, kind="ExternalInput")
    w2t_d = nc.dram_tensor("w2t", [D_FF, D_MODEL], F16, kind="ExternalInput")
    b1_d = nc.dram_tensor("b1c", [128, M1], F32, kind="ExternalInput")
    b2_d = nc.dram_tensor("b2c", [1, D_MODEL], F16, kind="ExternalInput")
    ones_d = nc.dram_tensor("ones", [1, 128], F16, kind="ExternalInput")
    bt_d = nc.dram_tensor("batcht", [D_MODEL, NC_ROWS], F16, kind="ExternalInput")
    b_d = nc.dram_tensor("batch", [NC_ROWS, D_MODEL], F32, kind="ExternalInput")
    out_d = nc.dram_tensor("out", [NC_ROWS, 1], F32, kind="ExternalOutput")

    n_groups = n_blocks * RG
    W1CB = 4                       # w1t column blocks (of 1024 d_ff each)

    with tile.TileContext(nc) as tc, ExitStack() as ctx:
        wpool = ctx.enter_context(tc.tile_pool(name="weights", bufs=1))
        btpool = ctx.enter_context(tc.tile_pool(name="batchT", bufs=16))
        ipool = ctx.enter_context(tc.tile_pool(name="innerT", bufs=36))
        bfpool = ctx.enter_context(tc.tile_pool(name="batchf", bufs=4))
        wxpool = ctx.enter_context(tc.tile_pool(name="wx", bufs=4))
        spool = ctx.enter_context(tc.tile_pool(name="scratch", bufs=2))
        zpool = ctx.enter_context(tc.tile_pool(name="z", bufs=1))
        psum1 = ctx.enter_context(tc.tile_pool(name="psum1", bufs=3, space="PSUM"))
        psum2 = ctx.enter_context(tc.tile_pool(name="psum2", bufs=3, space="PSUM"))

        # DMA emission order = queue order: block-0 activations and the first
        # w1t column block go first so PE can start ~15us in; the remaining
        # weight bulk streams behind them.
        ones = wpool.tile([1, 128], F16, tag="ones")
        nc.sync.dma_start(ones[:], ones_d.ap()[:])

        # Warm the PE HAM clock gate during the DMA-bound startup window:
        # ~150 tiny N=64 matmuls (~7us) starting when the 256B ones tile
        # lands (~10us) and ending as the first real operands arrive
        # (~17us), so the real stream issues at K=8/8 from its first op.
        wpsum = ctx.enter_context(tc.tile_pool(name="wpsum", bufs=1, space="PSUM"))
        warm = wpsum.tile([64, 64], F32)
        for _ in range(150):
            nc.tensor.matmul(
                warm[:], ones[:, 0:64], ones[:, 0:64], start=True, stop=True
            )

        bt0 = []
        for k in range(K1):
            t = btpool.tile([128, R], F16, tag="bt")
            nc.sync.dma_start(t[:], bt_d.ap()[k * 128:(k + 1) * 128, 0:R])
            bt0.append(t)

        CBW = D_FF // W1CB
        w1t = [[None] * W1CB for _ in range(K1)]
        for k in range(K1):
            t = wpool.tile([128, CBW], F16, tag=f"w1t{k}c0")
            nc.sync.dma_start(t[:], w1t_d.ap()[k * 128:(k + 1) * 128, 0:CBW])
            w1t[k][0] = t

        b1t = wpool.tile([128, M1], F32, tag="b1t")
        nc.sync.dma_start(b1t[:], b1_d.ap()[:])
        b2t = wpool.tile([1, D_MODEL], F16, tag="b2t")
        nc.sync.dma_start(b2t[:], b2_d.ap()[:])

        # rest of W1T column blocks
        for cb in range(1, W1CB):
            for k in range(K1):
                t = wpool.tile([128, CBW], F16, tag=f"w1t{k}c{cb}")
                nc.sync.dma_start(
                    t[:], w1t_d.ap()[k * 128:(k + 1) * 128, cb * CBW:(cb + 1) * CBW]
                )
                w1t[k][cb] = t
        # W2T chunks (first needed ~45us in, at phase 2 of block 0)
        w2t = []
        for m in range(M1):
            t = wpool.tile([128, D_MODEL], F16, tag=f"w2t{m}")
            nc.sync.dma_start(t[:], w2t_d.ap()[m * 128:(m + 1) * 128, :])
            w2t.append(t)

        z_all = zpool.tile([128, n_groups], F32)
        sig = zpool.tile([128, n_groups], F32, tag="sig")

        for b in range(n_blocks):
            # batchT chunks for this row block
            if b == 0:
                bt = bt0
            else:
                bt = []
                for k in range(K1):
                    t = btpool.tile([128, R], F16, tag="bt")
                    nc.sync.dma_start(
                        t[:], bt_d.ap()[k * 128:(k + 1) * 128, b * R:(b + 1) * R]
                    )
                    bt.append(t)

            # phase 1: innerT chunks [128 dff, R rows]
            it = []
            for m in range(M1):
                cb, mo = divmod(m, CBW // 128)
                ps = psum1.tile([128, R], F32)
                for k in range(K1):
                    nc.tensor.matmul(
                        ps[:],
                        w1t[k][cb][:, mo * 128:(mo + 1) * 128],
                        bt[k][:],
                        start=(k == 0),
                        stop=(k == K1 - 1),
                    )
                t = ipool.tile([128, R], F16, tag="it")
                nc.scalar.activation(
                    t[:], ps[:], mybir.ActivationFunctionType.Tanh,
                    bias=b1t[:, m:m + 1],
                )
                it.append(t)

            # phase 2 + row-dot per 128-row group
            for rg in range(RG):
                g = b * RG + rg
                bf = bfpool.tile([128, D_MODEL], F32, tag="bf")
                nc.sync.dma_start(
                    bf[:], b_d.ap()[g * 128:(g + 1) * 128, :]
                )
                wx = wxpool.tile([128, D_MODEL], F32, tag="wx")
                for h in range(NH):
                    ps2 = psum2.tile([128, 512], F32)
                    nc.tensor.matmul(
                        ps2[:],
                        ones[:],
                        b2t[:, h * 512:(h + 1) * 512],
                        start=True,
                        stop=False,
                    )
                    for m in range(M1):
                        nc.tensor.matmul(
                            ps2[:],
                            it[m][:, rg * 128:(rg + 1) * 128],
                            w2t[m][:, h * 512:(h + 1) * 512],
                            start=False,
                            stop=(m == M1 - 1),
                        )
                    nc.scalar.activation(
                        wx[:, h * 512:(h + 1) * 512], ps2[:],
                        mybir.ActivationFunctionType.Tanh,
                    )
                # z[g] = sum(wx * batch) along d_model, fused mult+reduce on DVE
                scratch = spool.tile([128, D_MODEL], F32, tag="scr")
                nc.vector.scalar_tensor_tensor(
                    out=scratch[:],
                    in0=wx[:],
                    scalar=1.0,
                    in1=bf[:],
                    op0=mybir.AluOpType.mult,
                    op1=mybir.AluOpType.mult,
                    accum_out=z_all[:, g:g + 1],
                )
                nc.scalar.activation(
                    sig[:, g:g + 1], z_all[:, g:g + 1],
                    mybir.ActivationFunctionType.Sigmoid,
                )
                nc.sync.dma_start(
                    out_d.ap()[g * 128:(g + 1) * 128, :], sig[:, g:g + 1]
                )

    return nc


_CACHED = {}


def _get_nc(n_blocks=N_BLOCKS):
    if n_blocks not in _CACHED:
        _CACHED[n_blocks] = build_bass(n_blocks)
    return _CACHED[n_blocks]


def _prep_in_maps(batch, W1, b1, W2, b2):
    batch = np.ascontiguousarray(batch, dtype=np.float32)
    w1t = np.ascontiguousarray(W1.T, dtype=np.float16)      # [1024, 4096]
    w2t = np.ascontiguousarray(W2.T, dtype=np.float16)      # [4096, 1024]
    # b1 as [128, 32]: column m holds b1[m*128:(m+1)*128] (per-partition bias)
    b1c = np.ascontiguousarray(
        np.asarray(b1, dtype=np.float32).reshape(M1, 128).T
    )
    b2c = np.ascontiguousarray(b2, dtype=np.float16).reshape(1, D_MODEL)
    ones = np.ones((1, 128), dtype=np.float16)
    batcht = np.ascontiguousarray(batch.T.astype(np.float16))  # [1024, 16384]

    in_maps = []
    for c in range(N_CORES):
        r0, r1 = c * NC_ROWS, (c + 1) * NC_ROWS
        in_maps.append({
            "w1t": w1t,
            "w2t": w2t,
            "b1c": b1c,
            "b2c": b2c,
            "ones": ones,
            "batcht": np.ascontiguousarray(batcht[:, r0:r1]),
            "batch": np.ascontiguousarray(batch[r0:r1]),
        })
    return in_maps


def kernel(batch, W1, b1, W2, b2, _trace=False, _trace_kwargs=None):
    in_maps = _prep_in_maps(batch, W1, b1, W2, b2)
    nc = _get_nc()
    res = bass_utils.run_bass_kernel_spmd(
        nc, in_maps, core_ids=list(range(N_CORES)),
        trace=_trace, **(_trace_kwargs or {}),
    )
    out = np.concatenate([res.results[c]["out"][:, 0] for c in range(N_CORES)])
    if _trace:
        return out, res
    return out
